# revision 17
# baseline (speedup 1.0000x reference)
"""CompoundHeadAttention TRN2 kernel (v3).

Full-input contract: kernel(**inputs) takes the unsharded tensors from
setup_inputs() and returns the full [1, 2048, 2048] float32 output.

Sharding (8 cores, tensor-parallel over the HK=8 kv heads):
  core h owns kv head h; computes its head's attention + its partial FC
  output [2048, 2048] fp16; the host sums the 8 partials and adds the
  (algebraically folded) bias terms.

Bias algebra (device carries NO bq/bk/bv):
  - bq folds into bG_eff = bq_h @ WG[h] + bG[h]  (host precompute)
  - bk adds a per-query constant to every logit -> cancels in softmax
  - bv contributes 1 (x) tile(bv_h, G) to hidden -> folded host-side
    into bfc_eff = bfc + sum_h tile(bv_h,4) @ Wfc[256h:256h+256]

v3 changes vs v2 (251us traced):
  - window-granular input tiles [128, 16, 512] (one DMA each, 2 MB),
    ALL input DMAs issued up-front on the sync queue in consumption
    order; V-transposes ride the scalar queue; out writes on sync after
    inputs.  No drip-fed DMA generators.
  - dedicated PSUM tags: proj accumulators get their own banks ("qv"
    packs Q rows 0:64 + V rows 64:128; "kp" holds K); attention "st"
    ring [128,1024] x2 is shared only with FC matmuls (both release
    fast), "pv" x2.  v2 aliased proj accumulators with the st ring,
    which serialized whole phases (10us PE gaps in the trace).
  - FC uses 1024-wide rhs (16 matmuls/window instead of 32).
  - psum->sbuf copies moved off the scalar engine (Pool/DVE) so ACT
    does only exp + the G-bias copies; per-window schedule interleaves
    attn(w) with proj(w+1) and fc(w-1) to keep PE occupancy high.
"""

import os
import sys

import numpy as np

if "/opt/trn_rl_repo" not in sys.path and os.path.isdir("/opt/trn_rl_repo"):
    sys.path.insert(0, "/opt/trn_rl_repo")

import concourse.bass as bass  # noqa: E402
import concourse.mybir as mybir  # noqa: E402
import concourse.tile as tile  # noqa: E402
from concourse import bacc  # noqa: E402
from concourse import bass_utils  # noqa: E402

F32 = mybir.dt.float32
F16 = mybir.dt.float16
BF16 = mybir.dt.bfloat16
AF = mybir.ActivationFunctionType

N = 2048
E = 2048
HK = 8
D = 64
G = 4
NB = 4         # 512-wide n-windows
W = 512        # window width


def build_program():
    nc = bacc.Bacc("TRN2", target_bir_lowering=False, debug=False,
                   enable_asserts=False)

    # ---- DRAM I/O ----
    # inputs per window w: [p, ec, t] = x^T[128*ec+p, 512*w+t]
    xq = nc.dram_tensor("xq", [NB, 128, 16, W], F16, kind="ExternalInput").ap()
    xk = nc.dram_tensor("xk", [NB, 128, 16, W], F16, kind="ExternalInput").ap()
    xv = nc.dram_tensor("xv", [NB, 128, 16, W], F16, kind="ExternalInput").ap()
    # weight chunk layouts: [128, 16*m] with e-chunk ec at cols [m*ec, m*ec+m)
    wq = nc.dram_tensor("wq", [128, 16 * 64], F16, kind="ExternalInput").ap()
    wk = nc.dram_tensor("wk", [128, 16 * 128], F16, kind="ExternalInput").ap()
    wv = nc.dram_tensor("wv", [128, 16 * 64], F16, kind="ExternalInput").ap()
    wg = nc.dram_tensor("wg", [64, 256], F16, kind="ExternalInput").ap()
    bg = nc.dram_tensor("bg", [128, 2], F32, kind="ExternalInput").ap()
    wfc = nc.dram_tensor("wfc", [256, E], F16, kind="ExternalInput").ap()
    out = nc.dram_tensor("out", [N, E], F16, kind="ExternalOutput").ap()

    with tile.TileContext(nc) as tc:
        build_tile_kernel(tc, xq=xq, xk=xk, xv=xv, wq=wq, wk=wk, wv=wv,
                          wg=wg, bg=bg, wfc=wfc, out=out)
    nc.compile()
    return nc


def build_tile_kernel(tc, *, xq, xk, xv, wq, wk, wv, wg, bg, wfc, out):
    nc = tc.nc

    import contextlib
    ctx = contextlib.ExitStack()
    ctx.__enter__()
    cp = ctx.enter_context(tc.tile_pool(name="persist", bufs=1))

    def ptile(shape, dtype, name):
        return cp.tile(shape, dtype, tag=name, name=name)

    # ---- persistent constants in SBUF ----
    wq_sb = ptile([128, 16 * 64], F16, "wq_sb")
    wk_sb = ptile([128, 16 * 128], F16, "wk_sb")
    wv_sb = ptile([128, 16 * 64], F16, "wv_sb")
    wg_sb = ptile([64, 256], F16, "wg_sb")
    wfc0_sb = ptile([128, E], F16, "wfc0_sb")
    wfc1_sb = ptile([128, E], F16, "wfc1_sb")
    bg_sb = ptile([128, 2], F32, "bg_sb")
    # causal mask constant: mask[s, n_local] = 1 if n_local >= s else 0,
    # duplicated side by side for the two g-halves of a pair
    mask_sb = ptile([128, 256], BF16, "mask_sb")
    nc.vector.memset(mask_sb[:], 1.0)
    # exp logit shift (cancels in softmax; keeps exp within fp16 range)
    eshift_sb = ptile([128, 1], F32, "eshift_sb")
    nc.vector.memset(eshift_sb[:], -35.0)
    mask3 = mask_sb[:].rearrange("p (h c) -> p h c", c=128)
    nc.gpsimd.affine_select(
        out=mask3, in_=mask3, compare_op=mybir.AluOpType.is_ge,
        fill=0.0, base=0, pattern=[[0, 2], [1, 128]], channel_multiplier=-1)
    # identity for PE-based V transpose
    ident_sb = ptile([64, 64], BF16, "ident_sb")
    nc.gpsimd.memset(ident_sb[:], 0.0)
    nc.gpsimd.affine_select(
        out=ident_sb[:], in_=ident_sb[:],
        compare_op=mybir.AluOpType.not_equal, fill=1.0, base=0,
        pattern=[[-1, 64]], channel_multiplier=1)
    # dummy broadcast: preloads the gpsimd pool config for
    # partition_broadcast off the critical path
    warm_sb = ptile([64, 1], F32, "warm_sb")
    nc.gpsimd.partition_broadcast(warm_sb[:], eshift_sb[0:1, 0:1])

    # per-window persistent activations
    kt_w = [ptile([128, W], F16, f"kt{j}") for j in range(NB)]
    # one tile per 128-token V chunk: DMA transpose writes at offset 0,
    # col 64 holds the ones column (softmax denominator row of PV)
    vo_w = [[ptile([128, 65], BF16, f"vo{j}_{c}") for c in range(4)]
            for j in range(NB)]
    for j in range(NB):
        for c in range(4):
            nc.vector.memset(vo_w[j][c][:, 64:65], 1.0)
    qg01_w = [ptile([128, W], F16, f"qg01_{j}") for j in range(NB)]
    qg23_w = [ptile([128, W], F16, f"qg23_{j}") for j in range(NB)]
    hid01_w = [ptile([128, W], F16, f"hid01_{j}") for j in range(NB)]
    hid23_w = [ptile([128, W], F16, f"hid23_{j}") for j in range(NB)]

    with ctx:
        in_pool = ctx.enter_context(tc.tile_pool(name="in_pool", bufs=2))
        qt_pool = ctx.enter_context(tc.tile_pool(name="qt_pool", bufs=2))
        pt_pool = ctx.enter_context(tc.tile_pool(name="pt_pool", bufs=4))
        rec_pool = ctx.enter_context(tc.tile_pool(name="rec_pool", bufs=2))
        stage_pool = ctx.enter_context(tc.tile_pool(name="stage", bufs=3))
        # PSUM banks: pj(1, sequential K/Q/V/G ring) + fc(1) +
        # st([128,1024] x2 = 4) + pv(2) = 8.  fc and proj each own a
        # bank so slow staging CASTs never stall the attention st ring.
        ps1 = ctx.enter_context(
            tc.tile_pool(name="ps1", bufs=1, space="PSUM"))
        ps2 = ctx.enter_context(
            tc.tile_pool(name="ps2", bufs=2, space="PSUM"))

        from itertools import chain as ichain

        xin = {"q": [None] * NB, "k": [None] * NB, "v": [None] * NB}

        def emit_dma_all():
            """weights + ALL input windows, queued up-front on sync."""
            nc.sync.dma_start(wk_sb[:], wk[:])
            order = [(0, "k"), ("wq", None), (0, "q"), ("wv", None),
                     (0, "v"), ("fc", None),
                     (1, "k"), (1, "q"), (1, "v"),
                     (2, "k"), (2, "q"), (2, "v"),
                     (3, "k"), (3, "q"), (3, "v")]
            srcs = {"q": xq, "k": xk, "v": xv}
            for w, t in order:
                if w == "wq":
                    nc.sync.dma_start(wq_sb[:], wq[:])
                    nc.sync.dma_start(wg_sb[:], wg[:])
                    nc.sync.dma_start(bg_sb[:], bg[:])
                    continue
                if w == "wv":
                    nc.sync.dma_start(wv_sb[:], wv[:])
                    continue
                if w == "fc":
                    nc.sync.dma_start(wfc0_sb[:], wfc[0:128, :])
                    nc.sync.dma_start(wfc1_sb[:], wfc[128:256, :])
                    continue
                tl = in_pool.tile([128, 16, W], F16, tag=f"x{t}",
                                  name=f"x{t}{w}")
                xin[t][w] = tl
                if w == 0:
                    for g in range(4):
                        nc.sync.dma_start(tl[:, 4 * g:4 * g + 4, :],
                                          srcs[t][w][:, 4 * g:4 * g + 4, :])
                else:
                    nc.sync.dma_start(tl[:], srcs[t][w])

        def emit_proj(w):
            """K -> Q -> G -> V for window w, one psum bank ("pj").
            G right after Q so the qg copies aren't HOL-blocked behind
            the V transposes on the scalar queue."""
            tq, tk, tv = xin["q"][w], xin["k"][w], xin["v"][w]
            kp_ps = ps1.tile([128, W], F32, tag="pj", name="kp_ps")
            for ec in range(16):
                nc.tensor.matmul(kp_ps[:], wk_sb[:, bass.ts(ec, 128)],
                                 tk[:, ec, :], start=(ec == 0),
                                 stop=(ec == 15))
                if ec % 2 == 1:
                    yield
            nc.vector.tensor_copy(kt_w[w][:], kp_ps[:])
            q_ps = ps1.tile([64, W], F32, tag="pj", name="q_ps")
            for ec in range(16):
                nc.tensor.matmul(q_ps[:], wq_sb[:, bass.ts(ec, 64)],
                                 tq[:, ec, :], start=(ec == 0),
                                 stop=(ec == 15))
                if ec % 2 == 1:
                    yield
            qt = qt_pool.tile([64, W], F16, tag="qt", name="qt")
            nc.vector.tensor_copy(qt[:], q_ps[:])
            g01 = ps1.tile([128, W], F32, tag="pj", name="g01")
            nc.tensor.matmul(g01[:], wg_sb[:, 0:128], qt[:],
                             start=True, stop=True)
            yield
            nc.scalar.activation(qg01_w[w][:], g01[:], AF.Identity,
                                 bias=bg_sb[:, 0:1])
            g23 = ps1.tile([128, W], F32, tag="pj", name="g23")
            nc.tensor.matmul(g23[:], wg_sb[:, 128:256], qt[:],
                             start=True, stop=True)
            yield
            nc.scalar.activation(qg23_w[w][:], g23[:], AF.Identity,
                                 bias=bg_sb[:, 1:2])
            v_ps = ps1.tile([64, W], F32, tag="pj", name="v_ps")
            for ec in range(16):
                nc.tensor.matmul(v_ps[:], wv_sb[:, bass.ts(ec, 64)],
                                 tv[:, ec, :], start=(ec == 0),
                                 stop=(ec == 15))
                if ec % 2 == 1:
                    yield
            vt = qt_pool.tile([64, W], BF16, tag="vt", name="vt")
            nc.vector.tensor_copy(vt[:], v_ps[:])
            # V transpose on PE (DMA transposes poison the SDMA fabric
            # with 256B packets while inputs stream)
            for c in range(4):
                tr = ps1.tile([128, 64], BF16, tag="pj", name="tr")
                nc.tensor.transpose(tr[:], vt[:, bass.ts(c, 128)],
                                    ident_sb[:])
                yield
                nc.vector.tensor_copy(vo_w[w][c][:, 0:64], tr[:])

        def emit_attn(j):
            klast = 4 * j + 3
            for p in range(2):  # g-pairs (2p, 2p+1)
                pv_a = ps2.tile([65, W], F32, tag="pv", name="pv_a")
                pv_b = ps2.tile([65, W], F32, tag="pv", name="pv_b")

                def flush(pend):
                    pt, k, off = pend
                    vsl = vo_w[k // 4][k % 4][:, 0:65]
                    nc.tensor.matmul(pv_a[:, off:W], vsl, pt[:, off:W],
                                     start=(k == 0), stop=(k == klast))
                    nc.tensor.matmul(pv_b[:, off:W], vsl,
                                     pt[:, W + off:2 * W],
                                     start=(k == 0), stop=(k == klast))

                qg = qg01_w[j] if p == 0 else qg23_w[j]
                pend = []
                for k in range(klast + 1):
                    kc = kt_w[k // 4][:, bass.ts(k % 4, 128)]
                    i = k - 4 * j
                    off = max(0, 128 * i)
                    st = ps2.tile([128, 2 * W], F32, tag="st", name="st")
                    nc.tensor.matmul(st[:, off:W], kc[0:64, :],
                                     qg[0:64, off:W],
                                     start=True, stop=True)
                    nc.tensor.matmul(st[:, W + off:2 * W], kc[64:128, :],
                                     qg[64:128, off:W],
                                     start=True, stop=True)
                    yield
                    pt = pt_pool.tile([128, 2 * W], BF16, tag="pt", name="pt")
                    st3 = st[:].rearrange("p (h c) -> p h c", c=W)
                    pt3 = pt[:].rearrange("p (h c) -> p h c", c=W)
                    # exp(8S - 35): the shift cancels in the softmax ratio
                    # and keeps all exp outputs within bf16 range
                    nc.scalar.activation(pt3[:, :, off:W],
                                         st3[:, :, off:W],
                                         AF.Exp, scale=8.0,
                                         bias=eshift_sb[:])
                    if i >= 0:
                        # zero out below-diagonal cols [off, off+128)
                        nc.vector.tensor_mul(pt3[:, :, off:off + 128],
                                             pt3[:, :, off:off + 128],
                                             mask3)
                    pend.append((pt, k, off))
                    if len(pend) > 2:   # 2-chunk lag hides exp+mask latency
                        flush(pend.pop(0))
                        yield
                while pend:
                    flush(pend.pop(0))
                    yield
                # normalize: hid[half] = pv[0:64] * 1/pv[64]
                hid = hid01_w[j] if p == 0 else hid23_w[j]
                for half, pv in ((0, pv_a), (1, pv_b)):
                    # custom-DVE recip can't read PSUM on HW: stage to SBUF
                    den = rec_pool.tile([1, W], F32, tag="den", name="den")
                    nc.vector.tensor_copy(den[:], pv[64:65, :])
                    rec = rec_pool.tile([1, W], F32, tag="rec", name="rec")
                    nc.vector.reciprocal_approx_fast(rec[:], den[:])
                    recr = rec_pool.tile([64, W], F32, tag="recr",
                                         name="recr")
                    nc.gpsimd.partition_broadcast(recr[:], rec[:])
                    nc.vector.tensor_mul(hid[half * 64:half * 64 + 64, :],
                                         pv[0:64, :], recr[:])

        def emit_fc(j, rings=("fc",)):
            i = 0
            for m in range(4):
                msl = bass.ts(m, 128)
                stage = stage_pool.tile([128, E], F16, tag="fco",
                                        name="stage")
                rows = slice(W * j + 128 * m, W * j + 128 * m + 128)
                for eo in range(4):
                    fc_ps = ps1.tile([128, W], F32, tag=rings[i % len(rings)],
                                     name="fc_ps")
                    i += 1
                    csl = bass.ts(eo, W)
                    nc.tensor.matmul(fc_ps[:], hid01_w[j][:, msl],
                                     wfc0_sb[:, csl],
                                     start=True, stop=False)
                    yield
                    nc.tensor.matmul(fc_ps[:], hid23_w[j][:, msl],
                                     wfc1_sb[:, csl],
                                     start=False, stop=True)
                    yield
                    if eo % 2 == 0:
                        nc.vector.tensor_copy(stage[:, csl], fc_ps[:])
                    else:
                        nc.scalar.copy(stage[:, csl], fc_ps[:])
                    if j == 3 and m == 3 and eo == 1:
                        nc.sync.dma_start(out[rows, 0:1024],
                                          stage[:, 0:1024])
                if j == 3 and m == 3:
                    nc.sync.dma_start(out[rows, 1024:2048],
                                      stage[:, 1024:2048])
                else:
                    nc.sync.dma_start(out[rows, :], stage[:])

        def drain(g):
            for _ in g:
                pass

        def rr(pairs):
            """round-robin emission: [(generator, steps_per_turn)]"""
            live = [[g, w] for g, w in pairs]
            while live:
                for gw in list(live):
                    g, w = gw
                    try:
                        for _ in range(w):
                            next(g)
                    except StopIteration:
                        live.remove(gw)

        emit_dma_all()
        # Emission (= dependency) order must respect attn(j) -> fc(j);
        # fc(j) overlaps attn(j+1) instead, and attn-heavy weights let
        # the fc/proj streams drain last to cover normalize stalls.
        drain(emit_proj(0))
        rr([(emit_attn(0), 2), (emit_proj(1), 1)])
        rr([(emit_attn(1), 3), (emit_fc(0), 1), (emit_proj(2), 1)])
        rr([(emit_attn(2), 3), (emit_fc(1), 1), (emit_proj(3), 1)])
        rr([(emit_attn(3), 3), (emit_fc(2), 1)])
        drain(emit_fc(3, rings=("fc", "pj")))


def shard_inputs(inputs):
    """full inputs -> list of 8 per-core in_maps (numpy, device layouts)"""
    f16 = np.float16
    f32 = np.float32
    q = np.asarray(inputs["q"], f32)[0]
    k = np.asarray(inputs["k"], f32)[0]
    v = np.asarray(inputs["v"], f32)[0]
    Wq = np.asarray(inputs["Wq"], f32)
    Wk = np.asarray(inputs["Wk"], f32)
    Wv = np.asarray(inputs["Wv"], f32)
    bq = np.asarray(inputs["bq"], f32)
    WG = np.asarray(inputs["WG"], f32)
    bG = np.asarray(inputs["bG"], f32)
    Wfc = np.asarray(inputs["Wfc"], f32)

    def winged(x):
        # x [N, E] -> xT [E, N] -> [w, p, ec, t]
        xt = x.T.astype(f16).reshape(16, 128, NB, W)
        return np.ascontiguousarray(xt.transpose(2, 1, 0, 3))

    xq = winged(q)
    xk = winged(k)
    xv = winged(v)

    def chunked(w):
        # [E, m] -> [128, 16*m]: e-chunk ec at cols [m*ec, m*ec+m)
        M = w.shape[1]
        return np.ascontiguousarray(
            w.reshape(16, 128, M).transpose(1, 0, 2).reshape(128, 16 * M))

    maps = []
    for h in range(HK):
        sl = slice(h * D, (h + 1) * D)
        wk_h = Wk[:, sl]
        # bG_eff = bq_h @ WG[h] + bG[h]  (bq folded into the G bias)
        bg_eff = (bq[sl] @ WG[h] + bG[h]).astype(f32)
        m = {
            "xq": xq, "xk": xk, "xv": xv,
            "wq": chunked(Wq[:, sl]).astype(f16),
            "wk": chunked(np.concatenate([wk_h, wk_h], 1)).astype(f16),
            "wv": chunked(Wv[:, sl]).astype(f16),
            "wg": WG[h].astype(f16),                        # [64, 256]
            "bg": np.ascontiguousarray(
                bg_eff.reshape(2, 128).T).astype(f32),      # [128, 2]
            "wfc": Wfc[h * 256:(h + 1) * 256, :].astype(f16),
        }
        maps.append(m)
    return maps


_compiled = None
last_results = None


def get_compiled():
    global _compiled
    if _compiled is None:
        _compiled = build_program()
    return _compiled


def kernel(**inputs):
    global last_results
    nc = get_compiled()
    in_maps = shard_inputs(inputs)
    last_results = bass_utils.run_bass_kernel_spmd(
        nc, in_maps, core_ids=list(range(8)))
    bfc = np.asarray(inputs["bfc"], np.float64)
    bv = np.asarray(inputs["bv"], np.float64)
    Wfc = np.asarray(inputs["Wfc"], np.float64)
    # bv folded: hidden_true = hidden_dev + 1 (x) tile(bv_h, 4)
    bfc_eff = bfc.copy()
    for h in range(HK):
        bfc_eff += np.tile(bv[h * D:(h + 1) * D], G) @ \
            Wfc[h * 256:(h + 1) * 256, :]
    acc = np.zeros((N, E), np.float64)
    for res in last_results.results:
        acc += res["out"].astype(np.float64)
    full = (acc + bfc_eff[None, :]).astype(np.float32)
    return full.reshape(1, N, E)
